# revision 41
# baseline (speedup 1.0000x reference)
"""Longformer attention Trainium2 kernel (8 NeuronCores, SPMD).

Sharding: data-parallel over batch (cores 0-3 -> batch 0, 4-7 -> batch 1),
head-parallel within a batch group (4 heads = 256 channels per core).
Each core: QKV projection for its head slice, banded+global attention,
out-projection partial; host sums the 4 partials per batch and adds the
bias terms (bo and the fold of bv through Wo).
"""

import numpy as np
import ml_dtypes

import concourse.bacc as bacc
import concourse.mybir as mybir
from concourse.tile import TileContext
from concourse.bass_utils import run_bass_kernel_spmd

S = 2048          # sequence length
D = 1024          # model dim
NH = 16           # total heads
DH = 64           # head dim
HPC = 4           # heads per core
CPB = 4           # cores per batch
WIN = 256         # attention window (2 blocks of 128)
NB = S // 128     # 16 query/key blocks
BF16 = mybir.dt.bfloat16
F32 = mybir.dt.float32

_CACHE = {}


def _band(qb):
    return list(range(max(0, qb - 2), min(NB - 1, qb + 2) + 1))


def _mask_id(qb, kb):
    # 0:M1 lower edge, 1:M1g (+global key row), 2:M2 upper edge, 3:M2g (+global query col)
    if kb == qb - 2:
        return 1 if kb == 0 else 0
    if kb == qb + 2:
        return 3 if qb == 0 else 2
    return None


def build_masks():
    ki = np.arange(128)[:, None]
    qi = np.arange(128)[None, :]
    m1 = (qi <= ki).astype(np.float32)          # kb == qb-2 : valid iff qi <= ki
    m2 = (ki <= qi).astype(np.float32)          # kb == qb+2 : valid iff ki <= qi
    m1g = m1.copy(); m1g[0, :] = 1.0            # global key k=0 row
    m2g = m2.copy(); m2g[:, 0] = 1.0            # global query q=0 col
    return np.stack([m1, m1g, m2, m2g]).astype(ml_dtypes.bfloat16)


def build_program():
    nc = bacc.Bacc("TRN2", target_bir_lowering=False, debug=False, num_devices=8)

    xT = nc.dram_tensor("xT", [D, S], BF16, kind="ExternalInput").ap()
    wq = nc.dram_tensor("wq", [D, 2 * 128], BF16, kind="ExternalInput").ap()
    wk = nc.dram_tensor("wk", [D, 2 * 128], BF16, kind="ExternalInput").ap()
    wv = nc.dram_tensor("wv", [D, 2 * 128], BF16, kind="ExternalInput").ap()
    wo = nc.dram_tensor("wo", [2, 128, D], BF16, kind="ExternalInput").ap()
    bqd = nc.dram_tensor("bq", [2, 128, 1], F32, kind="ExternalInput").ap()
    bkd = nc.dram_tensor("bk", [2, 128, 1], F32, kind="ExternalInput").ap()
    maskd = nc.dram_tensor("masks", [4, 128, 128], BF16, kind="ExternalInput").ap()
    y = nc.dram_tensor("y", [S, D], F32, kind="ExternalOutput").ap()

    with TileContext(nc) as tc:
        import contextlib
        with contextlib.ExitStack() as ctx, \
                nc.allow_low_precision(reason="bf16 attention interior by design"):
            sbw = ctx.enter_context(tc.tile_pool(name="sbw", bufs=1))
            sbx = ctx.enter_context(tc.tile_pool(name="sbx", bufs=1))
            sbqk = ctx.enter_context(tc.tile_pool(name="sbqk", bufs=1))
            sbes = ctx.enter_context(tc.tile_pool(name="sbes", bufs=16))
            sbsm = ctx.enter_context(tc.tile_pool(name="sbsm", bufs=4))
            sbbc = ctx.enter_context(tc.tile_pool(name="sbbc", bufs=4))
            psA = ctx.enter_context(tc.tile_pool(name="psA", bufs=2, space="PSUM"))
            psS = ctx.enter_context(tc.tile_pool(name="psS", bufs=2, space="PSUM"))
            psPV = ctx.enter_context(tc.tile_pool(name="psPV", bufs=2, space="PSUM"))
            
            # ---- load inputs (weights+x interleaved per e-chunk so the
            # ---- first QKV accumulation can start as soon as e=0 lands) ----
            xt, wqt, wkt, wvt = [], [], [], []
            for e in range(8):
                for wi, (lst, src, nm) in enumerate(((wqt, wq, "q"), (wkt, wk, "k"))):
                    t = sbw.tile([128, 256], BF16, tag=f"w{nm}{e}")
                    eng = nc.sync if (e + wi) % 2 else nc.gpsimd
                    eng.dma_start(out=t[:], in_=src[e * 128:(e + 1) * 128, :])
                    lst.append(t)
                t = sbx.tile([128, S], BF16, tag=f"xT{e}")
                # first quarter of the sequence: span 0 of QT/KT only needs
                # x columns 0-511, so compute starts after ~1MB
                eng = nc.sync if e % 2 == 0 else nc.gpsimd
                eng.dma_start(out=t[:, 0:512], in_=xT[e * 128:(e + 1) * 128, 0:512])
                xt.append(t)
            for e in range(8):
                eng = nc.sync if e % 2 else nc.gpsimd
                eng.dma_start(out=xt[e][:, 512:1024],
                              in_=xT[e * 128:(e + 1) * 128, 512:1024])
            wot = []
            for cc in range(2):
                t = sbw.tile([128, D], BF16, tag=f"wo{cc}")
                nc.sync.dma_start(out=t[:], in_=wo[cc, :, :])
                wot.append(t)
            bqt, bkt = [], []
            for cc in range(2):
                tq = sbw.tile([128, 1], F32, tag=f"bq{cc}")
                nc.sync.dma_start(out=tq[:], in_=bqd[cc, :, :])
                bqt.append(tq)
                tk = sbw.tile([128, 1], F32, tag=f"bk{cc}")
                nc.sync.dma_start(out=tk[:], in_=bkd[cc, :, :])
                bkt.append(tk)
            ones1 = sbw.tile([1, 128], BF16, tag="ones1")
            nc.vector.memset(ones1[:], 1.0)
            mt = []
            for i in range(4):
                t = sbw.tile([128, 128], BF16, tag=f"mask{i}")
                nc.sync.dma_start(out=t[:], in_=maskd[i, :, :])
                mt.append(t)
            for e in range(8):
                t = sbw.tile([128, 256], BF16, tag=f"wv{e}", name="wvt")
                eng = nc.sync if e % 2 else nc.gpsimd
                eng.dma_start(out=t[:], in_=wv[e * 128:(e + 1) * 128, :])
                wvt.append(t)
            for e in range(8):
                eng = nc.sync if e % 2 == 0 else nc.gpsimd
                eng.dma_start(out=xt[e][:, 1024:2048],
                              in_=xT[e * 128:(e + 1) * 128, 1024:2048])

            # ---- persistent intermediates ----
            QT = [sbqk.tile([128, S], BF16, tag=f"QT{c}", name=f"QT{c}") for c in range(2)]
            KT = [sbqk.tile([128, S], BF16, tag=f"KT{c}", name=f"KT{c}") for c in range(2)]
            Vo = [sbqk.tile([128, HPC * 65], BF16, tag=f"Vo{t}", name=f"Vo{t}") for t in range(NB)]
            AO = [sbqk.tile([128, S], BF16, tag=f"AO{c}", name=f"AO{c}") for c in range(2)]
            recips = [[sbqk.tile([1, 1024], BF16, tag=f"recip{hf}_{h}", name=f"recip{hf}_{h}")
                       for h in range(HPC)] for hf in range(2)]

            # ---- phase A: projections, emitted span-by-span so the
            # ---- attention pipeline can start after the first spans ----
            def emit_qkt_span(ts):
                sp = slice(ts * 512, (ts + 1) * 512)
                for cc in range(2):
                    pq = psA.tile([128, 512], F32, tag="psA", name="pq")
                    for e in range(8):
                        nc.tensor.matmul(pq[:], wqt[e][:, cc * 128:(cc + 1) * 128],
                                         xt[e][:, sp], start=(e == 0), stop=(e == 7))
                    # Q' = (x Wq + bq) / 8 : scale folded in, bias pre-scaled on host
                    nc.vector.tensor_scalar(QT[cc][:, sp], pq[:], 0.125, bqt[cc][:],
                                            mybir.AluOpType.mult, mybir.AluOpType.add)
                    pk = psA.tile([128, 512], F32, tag="psA", name="pk")
                    for e in range(8):
                        nc.tensor.matmul(pk[:], wkt[e][:, cc * 128:(cc + 1) * 128],
                                         xt[e][:, sp], start=(e == 0), stop=(e == 7))
                    nc.vector.tensor_scalar(KT[cc][:, sp], pk[:], bkt[cc][:], None,
                                            mybir.AluOpType.add)
            def emit_v(tb):
                pv = psA.tile([128, 256], F32, tag="psA", name="pv")
                for e in range(8):
                    nc.tensor.matmul(pv[:], xt[e][:, tb * 128:(tb + 1) * 128],
                                     wvt[e][:], start=(e == 0), stop=(e == 7))
                # scatter heads into [h*65 : h*65+64]; col h*65+64 gets ones
                outap = Vo[tb][:, 0:260].rearrange("p (h c) -> p h c", h=4)[:, :, 0:64]
                inap = pv[:].rearrange("p (h c) -> p h c", h=4)
                nc.scalar.activation(outap, inap, mybir.ActivationFunctionType.Copy)
                onesap = Vo[tb][:, 0:260].rearrange("p (h c) -> p h c", h=4)[:, :, 64:65]
                nc.vector.memset(onesap, 1.0)

            # ---- global key (k=0) score rows, batched 4 qb per exp ----
            # esgt[h][g] covers qb 4g..4g+3 as [1, 512]; only slices for qb>=3 used
            esgt = [[None] * 4 for _ in range(HPC)]
            def emit_esg(g):
                for h in range(HPC):
                    hp, r0 = h // 2, (h % 2) * 64
                    psg = psA.tile([128, 512], F32, tag="psA", name="psg")
                    for j in range(4):
                        qb = 4 * g + j
                        if qb < 3:
                            continue
                        nc.tensor.matmul(psg[0:1, j * 128:(j + 1) * 128],
                                         KT[hp][r0:r0 + 64, 0:1],
                                         QT[hp][r0:r0 + 64, qb * 128:(qb + 1) * 128],
                                         start=True, stop=True)
                    eg = sbsm.tile([1, 512], BF16, tag=f"esg{h}_{g}", name="eg")
                    lo = 3 if g == 0 else 0
                    nc.scalar.activation(eg[0:1, lo * 128:512], psg[0:1, lo * 128:512],
                                         mybir.ActivationFunctionType.Exp)
                    esgt[h][g] = eg

            emit_qkt_span(0)
            emit_esg(0)
            emit_qkt_span(1)
            emit_esg(1)

            # ---- banded attention; V tiles emitted just-in-time so exp work
            # ---- starts early; pair 0 (which needs all V for the global row)
            # ---- runs after pair 4
            pair_order = [1, 2, 3, 4, 0, 5, 6, 7]
            v_before = {1: range(0, 6), 2: range(6, 8), 3: range(8, 10),
                        4: range(10, 12), 0: range(12, 16)}
            for pair in pair_order:
                if pair == 2:
                    emit_qkt_span(2)
                    emit_esg(2)
                    emit_qkt_span(3)
                    emit_esg(3)
                for tb in v_before.get(pair, ()):
                    emit_v(tb)
                qb0 = pair * 2
                for hp in range(2):
                    # both heads of the pair together: adjacent S^T matmuls hit
                    # different PE row-groups (partitions 0-63 vs 64-127) and
                    # overlap in the array
                    ppvs, jobs = {}, {0: [], 1: []}
                    for h2 in range(2):
                        ppvs[h2] = psPV.tile([65, 256], F32, tag="ppv", name="ppv")
                    for sub in range(2):
                        qb = qb0 + sub
                        qs = slice(qb * 128, (qb + 1) * 128)
                        kbs = _band(qb)
                        w = len(kbs) * 128
                        pss, ess = {}, {}
                        for h2 in range(2):
                            pss[h2] = psS.tile([128, 1024], F32, tag="psS", name="ps")
                        for i, kb in enumerate(kbs):
                            for h2 in range(2):
                                r0 = h2 * 64
                                nc.tensor.matmul(pss[h2][:, i * 128:(i + 1) * 128],
                                                 KT[hp][r0:r0 + 64, kb * 128:(kb + 1) * 128],
                                                 QT[hp][r0:r0 + 64, qs],
                                                 start=True, stop=True)
                        for h2 in range(2):
                            h = hp * 2 + h2
                            es = sbes.tile([128, 1024], BF16, tag="es", name="es")
                            nc.scalar.activation(es[:, 0:w], pss[h2][:, 0:w],
                                                 mybir.ActivationFunctionType.Exp)
                            for i, kb in enumerate(kbs):
                                mid = _mask_id(qb, kb)
                                if mid is not None:
                                    sl = slice(i * 128, (i + 1) * 128)
                                    nc.vector.tensor_mul(es[:, sl], es[:, sl], mt[mid][:])
                            ess[h2] = es
                        for h2 in range(2):
                            h = hp * 2 + h2
                            r0 = h2 * 64
                            hs = slice(h * 65, h * 65 + 65)
                            ov = ppvs[h2][:, sub * 128:(sub + 1) * 128]
                            for i, kb in enumerate(kbs):
                                jobs[h2].append((Vo[kb][:, hs],
                                                 ess[h2][:, i * 128:(i + 1) * 128],
                                                 ov, i == 0, sub))
                            if qb >= 3:  # global key k=0 column
                                eg = esgt[h][qb // 4]
                                co = (qb % 4) * 128
                                jobs[h2].append((Vo[0][0:1, hs], eg[0:1, co:co + 128],
                                                 ov, False, sub))
                            if qb == 0:  # global query q=0 vs far keys
                                ps0 = psA.tile([128, 512], F32, tag="psA", name="ps0")
                                for i, kb in enumerate(range(3, NB)):
                                    nc.tensor.matmul(
                                        ps0[:, i:i + 1],
                                        KT[hp][r0:r0 + 64, kb * 128:(kb + 1) * 128],
                                        QT[hp][r0:r0 + 64, 0:1], start=True, stop=True)
                                es0 = sbsm.tile([128, 13], BF16, tag="es0", name="es0")
                                nc.scalar.activation(es0[:], ps0[:, 0:13],
                                                     mybir.ActivationFunctionType.Exp)
                                for i, kb in enumerate(range(3, NB)):
                                    jobs[h2].append((Vo[kb][:, hs], es0[:, i:i + 1],
                                                     ppvs[h2][:, 0:1], False, sub))
                    for h2 in range(2):
                        h = hp * 2 + h2
                        pv_jobs = jobs[h2]
                        last_of_sub = {s: max(i for i, j in enumerate(pv_jobs)
                                              if j[4] == s) for s in (0, 1)}
                        for i_mm, (lh, rh, ov, first, sub) in enumerate(pv_jobs):
                            nc.tensor.matmul(ov, lh, rh, start=first,
                                             stop=(i_mm == last_of_sub[sub]))
                        qsp = slice(qb0 * 128, (qb0 + 2) * 128)
                        r0 = h2 * 64
                        nc.vector.tensor_copy(AO[hp][r0:r0 + 64, qsp],
                                              ppvs[h2][0:64, :])
                        nc.vector.reciprocal(
                            recips[qb0 // 8][h][0:1, qb0 % 8 * 128:(qb0 % 8 + 2) * 128],
                            ppvs[h2][64:65, :])

                # ---- normalize + out projection for this pair (spreads the
                # ---- output DMA across the whole attention phase) ----
                half, off = pair // 4, (qb0 % 8) * 128
                psp = slice(qb0 * 128, (qb0 + 2) * 128)
                for h in range(HPC):
                    hp, r0 = h // 2, (h % 2) * 64
                    pb = psA.tile([128, 512], F32, tag="psA", name="pb")
                    nc.tensor.matmul(pb[:, 0:256], ones1[0:1, :],
                                     recips[half][h][0:1, off:off + 256],
                                     start=True, stop=True)
                    bcs = sbbc.tile([128, 512], BF16, tag="bc", name="bcs")
                    nc.vector.tensor_copy(bcs[:, 0:256], pb[:, 0:256])
                    nc.vector.tensor_mul(AO[hp][r0:r0 + 64, psp],
                                         AO[hp][r0:r0 + 64, psp], bcs[r0:r0 + 64, 0:256])
                for qb2 in (qb0, qb0 + 1):
                    q2 = slice(qb2 * 128, (qb2 + 1) * 128)
                    for eh in range(2):
                        po = psA.tile([128, 512], F32, tag="psA", name="po")
                        for cc in range(2):
                            nc.tensor.matmul(po[:], AO[cc][:, q2],
                                             wot[cc][:, eh * 512:(eh + 1) * 512],
                                             start=(cc == 0), stop=(cc == 1))
                        ys = sbbc.tile([128, 512], F32, tag="ystage", name="ys")
                        if eh == 0:
                            nc.scalar.activation(ys[:], po[:],
                                                 mybir.ActivationFunctionType.Copy)
                        else:
                            nc.vector.tensor_copy(ys[:], po[:])
                        eng = nc.sync if (qb2 + eh) % 2 else nc.gpsimd
                        eng.dma_start(out=y[q2, eh * 512:(eh + 1) * 512], in_=ys[:])

    nc.compile()
    return nc


def kernel(x, Wq, bq, Wk, bk, Wv, bv, Wo, bo):
    x = np.asarray(x); Wq = np.asarray(Wq); bq = np.asarray(bq)
    Wk = np.asarray(Wk); bk = np.asarray(bk); Wv = np.asarray(Wv)
    bv = np.asarray(bv); Wo = np.asarray(Wo); bo = np.asarray(bo)
    if "nc" not in _CACHE:
        _CACHE["nc"] = build_program()
    nc = _CACHE["nc"]

    B = x.shape[0]
    masks = build_masks()
    bf = ml_dtypes.bfloat16
    in_maps = []
    for c in range(8):
        b = c // CPB
        h0 = (c % CPB) * HPC * DH          # channel offset of this core's heads
        sl = slice(h0, h0 + HPC * DH)
        in_maps.append({
            "xT": np.ascontiguousarray(x[b].T).astype(bf),
            "wq": np.ascontiguousarray(Wq[:, sl]).astype(bf),
            "wk": np.ascontiguousarray(Wk[:, sl]).astype(bf),
            "wv": np.ascontiguousarray(Wv[:, sl]).astype(bf),
            "wo": np.ascontiguousarray(Wo[sl, :]).reshape(2, 128, D).astype(bf),
            "bq": (bq[sl] * 0.125).reshape(2, 128, 1).astype(np.float32),
            "bk": bk[sl].reshape(2, 128, 1).astype(np.float32),
            "masks": masks,
        })
    res = run_bass_kernel_spmd(nc, in_maps, list(range(8)))
    out = np.zeros((B, S, D), dtype=np.float32)
    for c in range(8):
        out[c // CPB] += res.results[c]["y"]
    out += (bv @ Wo + bo)[None, None, :]
    return out


# revision 42
# speedup vs baseline: 1.0285x; 1.0285x over previous
"""Longformer attention Trainium2 kernel (8 NeuronCores, SPMD).

Sharding: data-parallel over batch (cores 0-3 -> batch 0, 4-7 -> batch 1),
head-parallel within a batch group (4 heads = 256 channels per core).
Each core: QKV projection for its head slice, banded+global attention,
out-projection partial; host sums the 4 partials per batch and adds the
bias terms (bo and the fold of bv through Wo).
"""

import numpy as np
import ml_dtypes

import concourse.bacc as bacc
import concourse.mybir as mybir
from concourse.tile import TileContext
from concourse.bass_utils import run_bass_kernel_spmd

S = 2048          # sequence length
D = 1024          # model dim
NH = 16           # total heads
DH = 64           # head dim
HPC = 4           # heads per core
CPB = 4           # cores per batch
WIN = 256         # attention window (2 blocks of 128)
NB = S // 128     # 16 query/key blocks
BF16 = mybir.dt.bfloat16
F32 = mybir.dt.float32

_CACHE = {}


def _band(qb):
    return list(range(max(0, qb - 2), min(NB - 1, qb + 2) + 1))


def _mask_id(qb, kb):
    # 0:M1 lower edge, 1:M1g (+global key row), 2:M2 upper edge, 3:M2g (+global query col)
    if kb == qb - 2:
        return 1 if kb == 0 else 0
    if kb == qb + 2:
        return 3 if qb == 0 else 2
    return None


def build_masks():
    ki = np.arange(128)[:, None]
    qi = np.arange(128)[None, :]
    m1 = (qi <= ki).astype(np.float32)          # kb == qb-2 : valid iff qi <= ki
    m2 = (ki <= qi).astype(np.float32)          # kb == qb+2 : valid iff ki <= qi
    m1g = m1.copy(); m1g[0, :] = 1.0            # global key k=0 row
    m2g = m2.copy(); m2g[:, 0] = 1.0            # global query q=0 col
    return np.stack([m1, m1g, m2, m2g]).astype(ml_dtypes.bfloat16)


def build_program():
    nc = bacc.Bacc("TRN2", target_bir_lowering=False, debug=False, num_devices=8)

    xT = nc.dram_tensor("xT", [D, S], BF16, kind="ExternalInput").ap()
    wq = nc.dram_tensor("wq", [D, 2 * 128], BF16, kind="ExternalInput").ap()
    wk = nc.dram_tensor("wk", [D, 2 * 128], BF16, kind="ExternalInput").ap()
    wv = nc.dram_tensor("wv", [D, 2 * 128], BF16, kind="ExternalInput").ap()
    wo = nc.dram_tensor("wo", [2, 128, D], BF16, kind="ExternalInput").ap()
    bqd = nc.dram_tensor("bq", [2, 128, 1], F32, kind="ExternalInput").ap()
    bkd = nc.dram_tensor("bk", [2, 128, 1], F32, kind="ExternalInput").ap()
    maskd = nc.dram_tensor("masks", [4, 128, 128], BF16, kind="ExternalInput").ap()
    y = nc.dram_tensor("y", [S, D], F32, kind="ExternalOutput").ap()

    with TileContext(nc) as tc:
        import contextlib
        with contextlib.ExitStack() as ctx, \
                nc.allow_low_precision(reason="bf16 attention interior by design"):
            sbw = ctx.enter_context(tc.tile_pool(name="sbw", bufs=1))
            sbx = ctx.enter_context(tc.tile_pool(name="sbx", bufs=1))
            sbqk = ctx.enter_context(tc.tile_pool(name="sbqk", bufs=1))
            sbes = ctx.enter_context(tc.tile_pool(name="sbes", bufs=16))
            sbsm = ctx.enter_context(tc.tile_pool(name="sbsm", bufs=4))
            sbbc = ctx.enter_context(tc.tile_pool(name="sbbc", bufs=4))
            psA = ctx.enter_context(tc.tile_pool(name="psA", bufs=2, space="PSUM"))
            psS = ctx.enter_context(tc.tile_pool(name="psS", bufs=2, space="PSUM"))
            psPV = ctx.enter_context(tc.tile_pool(name="psPV", bufs=2, space="PSUM"))
            
            # ---- load inputs (weights+x interleaved per e-chunk so the
            # ---- first QKV accumulation can start as soon as e=0 lands) ----
            xt, wqt, wkt, wvt = [], [], [], []
            for e in range(8):
                for wi, (lst, src, nm) in enumerate(((wqt, wq, "q"), (wkt, wk, "k"))):
                    t = sbw.tile([128, 256], BF16, tag=f"w{nm}{e}")
                    eng = nc.sync if (e + wi) % 2 else nc.gpsimd
                    eng.dma_start(out=t[:], in_=src[e * 128:(e + 1) * 128, :])
                    lst.append(t)
                t = sbx.tile([128, S], BF16, tag=f"xT{e}")
                # first quarter of the sequence: span 0 of QT/KT only needs
                # x columns 0-511, so compute starts after ~1MB
                eng = nc.sync if e % 2 == 0 else nc.gpsimd
                eng.dma_start(out=t[:, 0:512], in_=xT[e * 128:(e + 1) * 128, 0:512])
                xt.append(t)
            for e in range(8):
                eng = nc.sync if e % 2 else nc.gpsimd
                eng.dma_start(out=xt[e][:, 512:1024],
                              in_=xT[e * 128:(e + 1) * 128, 512:1024])
            wot = []
            for cc in range(2):
                t = sbw.tile([128, D], BF16, tag=f"wo{cc}")
                nc.sync.dma_start(out=t[:], in_=wo[cc, :, :])
                wot.append(t)
            bqt, bkt = [], []
            for cc in range(2):
                tq = sbw.tile([128, 1], F32, tag=f"bq{cc}")
                nc.sync.dma_start(out=tq[:], in_=bqd[cc, :, :])
                bqt.append(tq)
                tk = sbw.tile([128, 1], F32, tag=f"bk{cc}")
                nc.sync.dma_start(out=tk[:], in_=bkd[cc, :, :])
                bkt.append(tk)
            ones1 = sbw.tile([1, 128], BF16, tag="ones1")
            nc.vector.memset(ones1[:], 1.0)
            mt = []
            for i in range(4):
                t = sbw.tile([128, 128], BF16, tag=f"mask{i}")
                nc.sync.dma_start(out=t[:], in_=maskd[i, :, :])
                mt.append(t)
            for e in range(8):
                t = sbw.tile([128, 256], BF16, tag=f"wv{e}", name="wvt")
                eng = nc.sync if e % 2 else nc.gpsimd
                eng.dma_start(out=t[:], in_=wv[e * 128:(e + 1) * 128, :])
                wvt.append(t)
            for e in range(8):
                eng = nc.sync if e % 2 == 0 else nc.gpsimd
                eng.dma_start(out=xt[e][:, 1024:2048],
                              in_=xT[e * 128:(e + 1) * 128, 1024:2048])

            # ---- persistent intermediates ----
            QT = [sbqk.tile([128, S], BF16, tag=f"QT{c}", name=f"QT{c}") for c in range(2)]
            KT = [sbqk.tile([128, S], BF16, tag=f"KT{c}", name=f"KT{c}") for c in range(2)]
            Vo = [sbqk.tile([128, HPC * 65], BF16, tag=f"Vo{t}", name=f"Vo{t}") for t in range(NB)]
            AO = [sbqk.tile([128, S], BF16, tag=f"AO{c}", name=f"AO{c}") for c in range(2)]
            recips = [[sbqk.tile([1, 1024], BF16, tag=f"recip{hf}_{h}", name=f"recip{hf}_{h}")
                       for h in range(HPC)] for hf in range(2)]

            # ---- phase A: projections, emitted span-by-span so the
            # ---- attention pipeline can start after the first spans ----
            def emit_qkt_span(ts):
                sp = slice(ts * 512, (ts + 1) * 512)
                for cc in range(2):
                    pq = psA.tile([128, 512], F32, tag="psA", name="pq")
                    for e in range(8):
                        nc.tensor.matmul(pq[:], wqt[e][:, cc * 128:(cc + 1) * 128],
                                         xt[e][:, sp], start=(e == 0), stop=(e == 7))
                    # Q' = (x Wq + bq) / 8 : scale folded in, bias pre-scaled on host
                    nc.vector.tensor_scalar(QT[cc][:, sp], pq[:], 0.125, bqt[cc][:],
                                            mybir.AluOpType.mult, mybir.AluOpType.add)
                    pk = psA.tile([128, 512], F32, tag="psA", name="pk")
                    for e in range(8):
                        nc.tensor.matmul(pk[:], wkt[e][:, cc * 128:(cc + 1) * 128],
                                         xt[e][:, sp], start=(e == 0), stop=(e == 7))
                    nc.vector.tensor_scalar(KT[cc][:, sp], pk[:], bkt[cc][:], None,
                                            mybir.AluOpType.add)
            def emit_v(tb):
                pv = psA.tile([128, 256], F32, tag="psA", name="pv")
                for e in range(8):
                    nc.tensor.matmul(pv[:], xt[e][:, tb * 128:(tb + 1) * 128],
                                     wvt[e][:], start=(e == 0), stop=(e == 7))
                # scatter heads into [h*65 : h*65+64]; col h*65+64 gets ones
                outap = Vo[tb][:, 0:260].rearrange("p (h c) -> p h c", h=4)[:, :, 0:64]
                inap = pv[:].rearrange("p (h c) -> p h c", h=4)
                nc.scalar.activation(outap, inap, mybir.ActivationFunctionType.Copy)
                onesap = Vo[tb][:, 0:260].rearrange("p (h c) -> p h c", h=4)[:, :, 64:65]
                nc.vector.memset(onesap, 1.0)

            # ---- global key (k=0) score rows, batched 4 qb per exp ----
            # esgt[h][g] covers qb 4g..4g+3 as [1, 512]; only slices for qb>=3 used
            esgt = [[None] * 4 for _ in range(HPC)]
            def emit_esg(g):
                for h in range(HPC):
                    hp, r0 = h // 2, (h % 2) * 64
                    psg = psA.tile([128, 512], F32, tag="psA", name="psg")
                    for j in range(4):
                        qb = 4 * g + j
                        if qb < 3:
                            continue
                        nc.tensor.matmul(psg[0:1, j * 128:(j + 1) * 128],
                                         KT[hp][r0:r0 + 64, 0:1],
                                         QT[hp][r0:r0 + 64, qb * 128:(qb + 1) * 128],
                                         start=True, stop=True)
                    eg = sbsm.tile([1, 512], BF16, tag=f"esg{h}_{g}", name="eg")
                    lo = 3 if g == 0 else 0
                    nc.scalar.activation(eg[0:1, lo * 128:512], psg[0:1, lo * 128:512],
                                         mybir.ActivationFunctionType.Exp)
                    esgt[h][g] = eg

            emit_qkt_span(0)
            emit_esg(0)
            emit_qkt_span(1)
            emit_esg(1)

            # ---- banded attention; V tiles emitted just-in-time so exp work
            # ---- starts early; pair 0 (which needs all V for the global row)
            # ---- runs after pair 4
            pair_order = [1, 2, 3, 4, 0, 5, 6, 7]
            v_before = {1: range(0, 6), 2: range(6, 8), 3: range(8, 10),
                        4: range(10, 12), 0: range(12, 16)}
            for pair in pair_order:
                if pair == 2:
                    emit_qkt_span(2)
                    emit_esg(2)
                    emit_qkt_span(3)
                    emit_esg(3)
                for tb in v_before.get(pair, ()):
                    emit_v(tb)
                qb0 = pair * 2
                for hp in range(2):
                    # both heads of the pair together: adjacent S^T matmuls hit
                    # different PE row-groups (partitions 0-63 vs 64-127) and
                    # overlap in the array
                    ppvs, jobs = {}, {0: [], 1: []}
                    for h2 in range(2):
                        ppvs[h2] = psPV.tile([65, 256], F32, tag="ppv", name="ppv")
                    for sub in range(2):
                        qb = qb0 + sub
                        qs = slice(qb * 128, (qb + 1) * 128)
                        kbs = _band(qb)
                        w = len(kbs) * 128
                        pss, ess = {}, {}
                        for h2 in range(2):
                            pss[h2] = psS.tile([128, 1024], F32, tag="psS", name="ps")
                        for i, kb in enumerate(kbs):
                            for h2 in range(2):
                                r0 = h2 * 64
                                nc.tensor.matmul(pss[h2][:, i * 128:(i + 1) * 128],
                                                 KT[hp][r0:r0 + 64, kb * 128:(kb + 1) * 128],
                                                 QT[hp][r0:r0 + 64, qs],
                                                 start=True, stop=True)
                        for h2 in range(2):
                            h = hp * 2 + h2
                            es = sbes.tile([128, 1024], BF16, tag="es", name="es")
                            nc.scalar.activation(es[:, 0:w], pss[h2][:, 0:w],
                                                 mybir.ActivationFunctionType.Exp)
                            for i, kb in enumerate(kbs):
                                mid = _mask_id(qb, kb)
                                if mid is not None:
                                    sl = slice(i * 128, (i + 1) * 128)
                                    nc.vector.tensor_mul(es[:, sl], es[:, sl], mt[mid][:])
                            ess[h2] = es
                        for h2 in range(2):
                            h = hp * 2 + h2
                            r0 = h2 * 64
                            hs = slice(h * 65, h * 65 + 65)
                            ov = ppvs[h2][:, sub * 128:(sub + 1) * 128]
                            for i, kb in enumerate(kbs):
                                jobs[h2].append((Vo[kb][:, hs],
                                                 ess[h2][:, i * 128:(i + 1) * 128],
                                                 ov, i == 0, sub))
                            if qb >= 3:  # global key k=0 column
                                eg = esgt[h][qb // 4]
                                co = (qb % 4) * 128
                                jobs[h2].append((Vo[0][0:1, hs], eg[0:1, co:co + 128],
                                                 ov, False, sub))
                            if qb == 0:  # global query q=0 vs far keys
                                ps0 = psA.tile([128, 512], F32, tag="psA", name="ps0")
                                for i, kb in enumerate(range(3, NB)):
                                    nc.tensor.matmul(
                                        ps0[:, i:i + 1],
                                        KT[hp][r0:r0 + 64, kb * 128:(kb + 1) * 128],
                                        QT[hp][r0:r0 + 64, 0:1], start=True, stop=True)
                                es0 = sbsm.tile([128, 13], BF16, tag="es0", name="es0")
                                nc.scalar.activation(es0[:], ps0[:, 0:13],
                                                     mybir.ActivationFunctionType.Exp)
                                for i, kb in enumerate(range(3, NB)):
                                    jobs[h2].append((Vo[kb][:, hs], es0[:, i:i + 1],
                                                     ppvs[h2][:, 0:1], False, sub))
                    for h2 in range(2):
                        h = hp * 2 + h2
                        pv_jobs = jobs[h2]
                        last_of_sub = {s: max(i for i, j in enumerate(pv_jobs)
                                              if j[4] == s) for s in (0, 1)}
                        for i_mm, (lh, rh, ov, first, sub) in enumerate(pv_jobs):
                            nc.tensor.matmul(ov, lh, rh, start=first,
                                             stop=(i_mm == last_of_sub[sub]))
                        qsp = slice(qb0 * 128, (qb0 + 2) * 128)
                        r0 = h2 * 64
                        nc.vector.tensor_copy(AO[hp][r0:r0 + 64, qsp],
                                              ppvs[h2][0:64, :])
                        nc.vector.reciprocal(
                            recips[qb0 // 8][h][0:1, qb0 % 8 * 128:(qb0 % 8 + 2) * 128],
                            ppvs[h2][64:65, :])

                # ---- normalize + out projection for this pair (spreads the
                # ---- output DMA across the whole attention phase) ----
                half, off = pair // 4, (qb0 % 8) * 128
                psp = slice(qb0 * 128, (qb0 + 2) * 128)
                for h in range(HPC):
                    hp, r0 = h // 2, (h % 2) * 64
                    pb = psA.tile([128, 512], F32, tag="psA", name="pb")
                    nc.tensor.matmul(pb[:, 0:256], ones1[0:1, :],
                                     recips[half][h][0:1, off:off + 256],
                                     start=True, stop=True)
                    nc.vector.tensor_mul(AO[hp][r0:r0 + 64, psp],
                                         AO[hp][r0:r0 + 64, psp], pb[r0:r0 + 64, 0:256])
                for qb2 in (qb0, qb0 + 1):
                    q2 = slice(qb2 * 128, (qb2 + 1) * 128)
                    for eh in range(2):
                        po = psA.tile([128, 512], F32, tag="psA", name="po")
                        for cc in range(2):
                            nc.tensor.matmul(po[:], AO[cc][:, q2],
                                             wot[cc][:, eh * 512:(eh + 1) * 512],
                                             start=(cc == 0), stop=(cc == 1))
                        ys = sbbc.tile([128, 512], F32, tag="ystage", name="ys")
                        if eh == 0:
                            nc.scalar.activation(ys[:], po[:],
                                                 mybir.ActivationFunctionType.Copy)
                        else:
                            nc.vector.tensor_copy(ys[:], po[:])
                        eng = nc.sync if (qb2 + eh) % 2 else nc.gpsimd
                        eng.dma_start(out=y[q2, eh * 512:(eh + 1) * 512], in_=ys[:])

    nc.compile()
    return nc


def kernel(x, Wq, bq, Wk, bk, Wv, bv, Wo, bo):
    x = np.asarray(x); Wq = np.asarray(Wq); bq = np.asarray(bq)
    Wk = np.asarray(Wk); bk = np.asarray(bk); Wv = np.asarray(Wv)
    bv = np.asarray(bv); Wo = np.asarray(Wo); bo = np.asarray(bo)
    if "nc" not in _CACHE:
        _CACHE["nc"] = build_program()
    nc = _CACHE["nc"]

    B = x.shape[0]
    masks = build_masks()
    bf = ml_dtypes.bfloat16
    in_maps = []
    for c in range(8):
        b = c // CPB
        h0 = (c % CPB) * HPC * DH          # channel offset of this core's heads
        sl = slice(h0, h0 + HPC * DH)
        in_maps.append({
            "xT": np.ascontiguousarray(x[b].T).astype(bf),
            "wq": np.ascontiguousarray(Wq[:, sl]).astype(bf),
            "wk": np.ascontiguousarray(Wk[:, sl]).astype(bf),
            "wv": np.ascontiguousarray(Wv[:, sl]).astype(bf),
            "wo": np.ascontiguousarray(Wo[sl, :]).reshape(2, 128, D).astype(bf),
            "bq": (bq[sl] * 0.125).reshape(2, 128, 1).astype(np.float32),
            "bk": bk[sl].reshape(2, 128, 1).astype(np.float32),
            "masks": masks,
        })
    res = run_bass_kernel_spmd(nc, in_maps, list(range(8)))
    out = np.zeros((B, S, D), dtype=np.float32)
    for c in range(8):
        out[c // CPB] += res.results[c]["y"]
    out += (bv @ Wo + bo)[None, None, :]
    return out
